# revision 64
# baseline (speedup 1.0000x reference)
"""Trainium2 Bass kernel for nn_MultiHeadedAttention — transposed dataflow.

Scores are computed TRANSPOSED: S^T[k, q] = (c_k kd_hat).(a_q qd_hat), with all
norm/scale factors folded into the projected direction vectors (a = S*qn/|qd|,
c = S*kn/|kd|, S = 10/32^0.25), all in bf16. A per-query softmax shift m_q
rides the score matmul as an augmented contraction row (K=33): k-side aux row
= 1, q-side aux row = (B_SHIFT - m_q). m_q = LAM*|S*qn_q|*RMS_k(S*kn) is a
statistical upper bound on the row max (validated offline:
allmax_q - 85 <= m_q <= unmasked_max_q + 78), which keeps exp in range
(softmax is invariant to per-q shifts, so m_q only needs range-safety).

Scores land in bf16 PSUM (1 bank per [128,1024] tile). exp is split across two
engines to balance load:
 - ACT path: activation Exp with bias -B_SHIFT  -> e = exp(s - m_q) in bf16.
 - DVE path (Schraudolph bit-trick): bits_i16 = max(A_TRICK*s', 0) via ONE
   tensor_scalar (f32->i16 conversion saturates on TRN2; verified on HW).
   B_SHIFT = (127*128 - C_TRICK)/A_TRICK is pre-added by the aux row, so the
   int16 bits ARE bf16(exp) within +-3% multiplicative error (mean-centered
   via C_TRICK; softmax num/den ratio cancels most of it).
Masking is a bitwise AND with an int16 {0,-1} mask (exp>=0 so AND == select),
routed DVE (2x mode) or GPSIMD to balance. Numerator and denominator both
come from ONE PE matmul pair per tile: [num; den] = [v | 1]^T @ et.

Per-core cost model (TRN2 TimelineSim): PE ~470us, ACT/DVE/Pool ~380-400us.
Sharding: core c -> batch b=c//2, query-half c%2 (mask read exactly once).
"""

import ml_dtypes
import numpy as np

BF16NP = ml_dtypes.bfloat16

import concourse.bass as bass
import concourse.mybir as mybir
from concourse import bacc
from concourse.tile import TileContext
from concourse import bass_utils
from concourse.masks import make_identity

F32 = mybir.dt.float32
BF16 = mybir.dt.bfloat16
I16 = mybir.dt.int16

B, SQ, SK, D, H, DK = 4, 4096, 4096, 256, 8, 32
NCORES = 8
R = SQ // 2          # q rows per core
QH = R // 1024       # 2 q-half blocks of 1024
KT = SK // 128       # 32 k-tiles of 128
SCALE = 10.0 / (32.0 ** 0.25)
LAM = 1.51           # shift coefficient, window [1.36, 1.66] w/ margins (85,78)

A_TRICK = 128.0 / float(np.log(2.0))       # bf16 bits per unit of exp arg
C_TRICK = 5.0                              # mean-centering offset (bits)
B_SHIFT = (127.0 * 128.0 - C_TRICK) / A_TRICK   # pre-added to scores (~88.0)

# routing: which kc tiles (mod KT) use the DVE bit-trick exp, and which
# kc tiles get their mask-AND on GPSIMD (Pool) instead of DVE.
TRICK_N = 8          # of 32 tiles -> DVE bit-trick exp (rest ACT)
POOL_N = 8           # of 32 tiles -> Pool mask-mult (rest DVE)
TRICK_KCS = frozenset((i * KT) // TRICK_N + 2 for i in range(TRICK_N))
POOL_KCS = frozenset((3, 5, 9, 11, 15, 17, 21, 23))

_CACHE = {}


def _build(repeat=1, main_loop=True):
    key = (repeat, main_loop)
    if key in _CACHE:
        return _CACHE[key]
    nc = bacc.Bacc("TRN2", target_bir_lowering=False, debug=False,
                   num_devices=NCORES)

    q_d = nc.dram_tensor("q", [R, D], BF16, kind="ExternalInput")
    k_d = nc.dram_tensor("k", [SK, D], BF16, kind="ExternalInput")
    v_d = nc.dram_tensor("v", [1, SK], F32, kind="ExternalInput")
    mt_d = nc.dram_tensor("mt", [SK, R], BF16, kind="ExternalInput")
    # w0p: outc-permuted+padded w0.T -> [inc, 4 groups x 128]
    w0p_d = nc.dram_tensor("w0p", [D, 4 * 128], BF16, kind="ExternalInput")
    w1t8_d = nc.dram_tensor("w1t8", [D, H], BF16, kind="ExternalInput")
    b0v_d = nc.dram_tensor("b0v", [128, 4], F32, kind="ExternalInput")
    b18v_d = nc.dram_tensor("b18v", [H, 1], F32, kind="ExternalInput")
    inds_d = nc.dram_tensor("inds", [128, 4 * H], BF16, kind="ExternalInput")
    indst_d = nc.dram_tensor("indst", [H, 4 * 128], BF16, kind="ExternalInput")
    out_d = nc.dram_tensor("o", [QH, 1024], F32, kind="ExternalOutput")

    with TileContext(nc) as tc:
        with tc.tile_pool(name="persist", bufs=1) as pp:
            w0p = pp.tile([128, 2, 4, 128], BF16, tag="w0p")
            nc.sync.dma_start(w0p[:], w0p_d.rearrange("(a p) (g o) -> p a g o",
                                                      p=128, g=4))
            w1t8 = pp.tile([128, 2, H], BF16, tag="w1t8")
            nc.sync.dma_start(w1t8[:], w1t8_d.rearrange("(a p) o -> p a o", p=128))
            b0v = pp.tile([128, 4], F32, tag="b0v")
            nc.sync.dma_start(b0v[:], b0v_d[:])
            b18v = pp.tile([H, 1], F32, tag="b18v")
            nc.sync.dma_start(b18v[:], b18v_d[:])
            nbsh = pp.tile([128, 1], F32, tag="nbsh")
            nc.gpsimd.memset(nbsh[:], -B_SHIFT)
            b18s = pp.tile([H, 1], F32, tag="b18s")
            nc.vector.tensor_scalar(out=b18s[:], in0=b18v[:], scalar1=SCALE,
                                    scalar2=None, op0=mybir.AluOpType.mult)
            inds = pp.tile([128, 4, H], BF16, tag="inds")
            nc.sync.dma_start(inds[:], inds_d.rearrange("p (g o) -> p g o", g=4))
            indst = pp.tile([H, 4, 128], BF16, tag="indst")
            nc.sync.dma_start(indst[:], indst_d.rearrange("p (g o) -> p g o", g=4))

            # [v | 1] stationary operands for the PV matmul, per k-tile
            uvt = pp.tile([128, KT, 2], BF16, tag="uvt")
            nc.gpsimd.dma_start(uvt[:, :, 0],
                                v_d.rearrange("a (c p) -> p (a c)", p=128))
            nc.gpsimd.memset(uvt[:, :, 1:2], 1.0)

            # projected tensors, augmented layout:
            # group gp=h//2: head dims at rows 64*(h%2)..+32, aux row at 32/96
            qdT = pp.tile([128, 4, R], BF16, tag="qdT")
            kdT = pp.tile([128, 4, SK], BF16, tag="kdT")
            mall_ctx = tc.tile_pool(name="mall", bufs=1)
            mallp = mall_ctx.__enter__()
            shp_ctx = tc.tile_pool(name="shp", bufs=1)
            shp = shp_ctx.__enter__()
            mq = shp.tile([8, R], F32, tag="mq")         # SCALE*|qn|
            sskp = shp.tile([8, 8], F32, tag="sskp")     # per-chunk sum kn'^2

            SSK_CHUNKS = 4   # k-chunks used for the shift statistic (2048
                             # keys; RMS sample noise ~1.6% vs LAM margin 10%)

            def project(src_d, rows, xdT, pfx, is_q, chunks=None):
                nch = rows // 512
                if chunks is None:
                    chunks = range(nch)
                with (
                    tc.tile_pool(name=pfx + "xT", bufs=3) as xTp,
                    tc.tile_pool(name=pfx + "psP", bufs=1, space="PSUM") as psP,
                    tc.tile_pool(name=pfx + "psS", bufs=2, space="PSUM") as psS,
                    tc.tile_pool(name=pfx + "psE", bufs=2, space="PSUM") as psE,
                    tc.tile_pool(name=pfx + "sq", bufs=2) as sqp,
                    tc.tile_pool(name=pfx + "sm", bufs=2) as smp,
                ):
                    for ch in chunks:
                        cs = slice(ch * 512, (ch + 1) * 512)
                        # hardware XBAR transpose DMA: [512 tok, 256 inc] DRAM
                        # -> xT[inc%128, inc//128, tok] SBUF, 64 xbar tiles
                        xT = xTp.tile([128, 2, 512], BF16, tag="xT")
                        nc.sync.dma_start_transpose(
                            xT[:], src_d[ch * 512:(ch + 1) * 512, :])
                        # norms projection qn[8, 512] (bias folded into consumers)
                        pn = psS.tile([8, 512], F32, tag="pn")
                        for kc in range(2):
                            nc.tensor.matmul(pn[:], w1t8[:, kc, :], xT[:, kc, :],
                                             start=(kc == 0), stop=(kc == 1))
                        if is_q:
                            # mq = SCALE*|qn| = |SCALE*pn + SCALE*b18|
                            nc.scalar.activation(
                                mq[:, cs], pn[:],
                                mybir.ActivationFunctionType.Abs,
                                bias=b18s[:], scale=SCALE)
                        elif ch < SSK_CHUNKS:
                            sqn = smp.tile([8, 512], F32, tag="sqn", bufs=1)
                            nc.scalar.activation(
                                sqn[:], pn[:],
                                mybir.ActivationFunctionType.Square,
                                bias=b18v[:])
                            nc.vector.tensor_reduce(
                                sskp[:, ch:ch + 1], sqn[:],
                                axis=mybir.AxisListType.X,
                                op=mybir.AluOpType.add)
                        # per-group direction projections + scaling;
                        # pr released fast (one ACT op) for cross-chunk
                        # overlap; squares from prb on Pool (SBUF-only there)
                        prbs = [None] * 4
                        sq_ = [None] * 4
                        for gp in range(4):
                            pr = psP.tile([128, 512], F32, tag=f"pr{gp % 2}",
                                          name=f"pr{gp % 2}")
                            for kc in range(2):
                                nc.tensor.matmul(
                                    pr[:], w0p[:, kc, gp, :], xT[:, kc, :],
                                    start=(kc == 0), stop=(kc == 1))
                            sq_[gp] = sqp.tile([128, 512], BF16, tag=f"sq{gp}",
                                               name=f"sq{gp}")
                            nc.scalar.activation(
                                sq_[gp][:], pr[:],
                                mybir.ActivationFunctionType.Square,
                                bias=b0v[:, gp:gp + 1])
                            prbs[gp] = sqp.tile([128, 512], BF16,
                                                tag=f"prb{gp}",
                                                name=f"prb{gp}")
                            nc.scalar.activation(
                                prbs[gp][:], pr[:],
                                mybir.ActivationFunctionType.Identity,
                                bias=b0v[:, gp:gp + 1])
                        pss = psS.tile([8, 512], F32, tag="pss")
                        for gp in range(4):
                            nc.tensor.matmul(pss[:], inds[:, gp, :], sq_[gp][:],
                                             start=(gp == 0), stop=(gp == 3))
                        srt = smp.tile([8, 512], F32, tag="srt", bufs=1)
                        nc.scalar.activation(srt[:], pss[:],
                                             mybir.ActivationFunctionType.Sqrt,
                                             scale=1.0 / (SCALE * SCALE))
                        rn = smp.tile([8, 512], F32, tag="rn")
                        nc.vector.reciprocal_approx_fast(rn[:], srt[:])
                        # av = (pn + b18)*rn  (per-head scale, bf16)
                        av = smp.tile([8, 512], BF16, tag="av")
                        nc.vector.scalar_tensor_tensor(
                            out=av[:], in0=pn[:], scalar=b18v[:], in1=rn[:],
                            op0=mybir.AluOpType.add, op1=mybir.AluOpType.mult)
                        # broadcast av to the 32-row head blocks (PE), then
                        # xdT = prb * av_bcast (DVE reads the PSUM broadcast)
                        for gp in range(4):
                            pe = psE.tile([128, 512], F32, tag="pe")
                            nc.tensor.matmul(pe[:], indst[:, gp, :], av[:])
                            for u in range(2):
                                rs = slice(64 * u, 64 * u + 32)
                                nc.vector.tensor_mul(
                                    xdT[rs, gp, cs],
                                    prbs[gp][rs, :],
                                    pe[rs, :])

            # ---- main attention loop (transposed scores) ----
            LAG = 4    # PV matmuls trail scores by LAG tiles (global stream)
            # flat (qh, h, kc) software pipeline: PV matmuls trail by LAG
            # tiles ACROSS head boundaries so PE never drains; mask DMAs for
            # the next q-half prefetch during the current one (and qh=0's
            # during the projection phase, emitted before project()).
            seq_qh = [qh for _rep in range(repeat) for qh in range(QH)]
            if not main_loop:
                seq_qh = []
            mh_tiles = {}

            def load_masks(i):
                qh = seq_qh[i]
                q0 = qh * 1024
                lst = []
                for mh in range(2):
                    mt = mallp.tile([128, KT // 2, 1024], BF16,
                                    tag=f"mall{mh}", name=f"mall{mh}_{i}",
                                    bufs=2 if mh == 0 else 1)
                    k0 = mh * (SK // 2)
                    for c4 in range(4):
                        nc.gpsimd.dma_start(
                            mt[:, 4 * c4:4 * c4 + 4, :],
                            mt_d[k0 + 512 * c4:k0 + 512 * (c4 + 1),
                                 q0:q0 + 1024].rearrange(
                                "(c p) q -> p c q", p=128))
                    lst.append(mt)
                mh_tiles[i] = lst

            if seq_qh:
                load_masks(0)

            project(q_d, R, qdT, "q", True)
            project(k_d, SK, kdT, "k", False)

            # aux rows: k-side ones (DMA from a separate ones tile; engine
            # memset cannot target partition base 96)
            ones4k = shp.tile([1, SK], BF16, tag="ones4k")
            nc.gpsimd.memset(ones4k[:], 1.0)
            for gp in range(4):
                nc.sync.dma_start(kdT[32:33, gp, :], ones4k[:])
                nc.sync.dma_start(kdT[96:97, gp, :], ones4k[:])
            # shift: ssk -> T = LAM*sqrt(ssk/SK) per head; aux = B_SHIFT - mq*T
            ssk = shp.tile([8, 1], F32, tag="ssk")
            nc.vector.tensor_reduce(ssk[:], sskp[:, 0:SSK_CHUNKS],
                                    axis=mybir.AxisListType.X,
                                    op=mybir.AluOpType.add)
            tsh = shp.tile([8, 1], F32, tag="tsh")
            nc.scalar.activation(tsh[:], ssk[:],
                                 mybir.ActivationFunctionType.Sqrt,
                                 scale=LAM * LAM * SCALE * SCALE / (SSK_CHUNKS * 512.0))
            ntsh = shp.tile([8, 1], F32, tag="ntsh")
            nc.vector.tensor_scalar(out=ntsh[:], in0=tsh[:], scalar1=-1.0,
                                    scalar2=None, op0=mybir.AluOpType.mult)
            negmq = shp.tile([8, R], F32, tag="negmq")
            nc.vector.tensor_scalar(out=negmq[:], in0=mq[:], scalar1=ntsh[:],
                                    scalar2=B_SHIFT, op0=mybir.AluOpType.mult,
                                    op1=mybir.AluOpType.add)
            # distribute aux rows into qdT (partition moves via DMA, casts)
            for h in range(H):
                gp, u = divmod(h, 2)
                nc.gpsimd.dma_start(qdT[32 + 64 * u:33 + 64 * u, gp, :],
                                    negmq[h:h + 1, :])

            shp_ctx.__exit__(None, None, None)

            _mainctx = [tc.tile_pool(name="psSc", bufs=3, space="PSUM"),
                        tc.tile_pool(name="psNd", bufs=1, space="PSUM"),
                        tc.tile_pool(name="psO", bufs=1, space="PSUM"),
                        tc.tile_pool(name="ebuf", bufs=5),
                        tc.tile_pool(name="etl", bufs=10),
                        tc.tile_pool(name="sm2", bufs=1)]
            psc, psnd, pso, ebufp, etlp, sm2p = [c.__enter__()
                                                 for c in _mainctx]
            oneH = sm2p.tile([H, 1], BF16, tag="oneH", name="oneH")
            nc.gpsimd.memset(oneH[:], 1.0)

            tiles = [(i, h, kc) for i in range(len(seq_qh))
                     for h in range(H) for kc in range(KT)]
            nds_d = {}     # (i, h) -> nd PSUM tile
            ets = {}       # (i, h, kc) -> masked-exp tile
            pv_pending = []

            def pv_lag(kc):
                if kc == KT - 1:
                    return 2   # order guard already forces it last
                return 8 if kc in POOL_KCS else 3
            q8 = {}        # i -> (num8, den8)
            due_tails = []

            def issue_pv(i, h, kc):
                nd = nds_d[(i, h)]
                for j in range(2):
                    nc.tensor.matmul(
                        nd[32 * j:32 * j + 2, :],
                        uvt[:, kc, :],
                        ets[(i, h, kc)][:, j * 512:(j + 1) * 512],
                        start=(kc == 0), stop=(kc == KT - 1))
                del ets[(i, h, kc)]
                if kc != KT - 1:
                    return
                # head h of q-half i fully accumulated: extract num/den
                # (engine copy PSUM->SBUF, lane-aligning SBUF->SBUF DMAs)
                num8, den8 = q8[i]
                nds = sm2p.tile([34, 512], F32, tag="nds", bufs=2)
                nc.vector.tensor_copy(nds[:], nd[:])
                del nds_d[(i, h)]
                nc.sync.dma_start(num8[h:h + 1, 0:512], nds[0:1, :])
                nc.sync.dma_start(num8[h:h + 1, 512:1024], nds[32:33, :])
                nc.sync.dma_start(den8[h:h + 1, 0:512], nds[1:2, :])
                nc.sync.dma_start(den8[h:h + 1, 512:1024], nds[33:34, :])
                if h != H - 1:
                    return
                due_tails.append((i, cur_idx[0]))

            def emit_tail(i):
                # batched softmax tail for q-half i: x = mean_h num/den
                num8, den8 = q8[i]
                qh = seq_qh[i]
                r8 = sm2p.tile([H, 1024], F32, tag="r8")
                nc.vector.reciprocal_approx_fast(r8[:], den8[:])
                xh8 = sm2p.tile([H, 1024], BF16, tag="xh8")
                nc.vector.scalar_tensor_tensor(
                    out=xh8[:], in0=num8[:], scalar=1.0 / H, in1=r8[:],
                    op0=mybir.AluOpType.mult, op1=mybir.AluOpType.mult)
                outp = pso.tile([33, 512], F32, tag="outp")
                for j in range(2):
                    nc.tensor.matmul(outp[32 * j:32 * j + 1, :], oneH[:],
                                     xh8[:, j * 512:(j + 1) * 512])
                outs = sm2p.tile([33, 512], F32, tag="outs")
                nc.vector.tensor_copy(outs[:], outp[:])
                nc.sync.dma_start(out_d[qh:qh + 1, 0:512], outs[0:1, :])
                nc.sync.dma_start(out_d[qh:qh + 1, 512:1024], outs[32:33, :])

            DEFER = 32
            cur_idx = [0]
            for idx, (i, h, kc) in enumerate(tiles):
                cur_idx[0] = idx
                while due_tails and due_tails[0][1] + DEFER <= idx:
                    emit_tail(due_tails.pop(0)[0])
                qh = seq_qh[i]
                q0 = qh * 1024
                if h == 0 and kc == 0:
                    q8[i] = (sm2p.tile([H, 1024], F32, tag="num8",
                                       name=f"num8_{i}", bufs=2),
                             sm2p.tile([H, 1024], F32, tag="den8",
                                       name=f"den8_{i}", bufs=2))
                if h == 1 and kc == 0 and i + 1 < len(seq_qh):
                    load_masks(i + 1)   # prefetch next q-half's masks
                gp, u = divmod(h, 2)
                r0 = 64 * u
                if kc == 0:
                    # nd: num/den at rows 0-1 (j=0), 32-33 (j=1): 1 PSUM bank
                    nds_d[(i, h)] = psnd.tile([34, 512], F32, tag="nd",
                                               name=f"nd_{i}_{h}")
                ps = psc.tile([128, 1024], F32, tag="ps")
                lhsT = kdT[r0:r0 + 33, gp, kc * 128:(kc + 1) * 128]
                for j in range(2):
                    nc.tensor.matmul(
                        ps[:, j * 512:(j + 1) * 512], lhsT,
                        qdT[r0:r0 + 33, gp, q0 + j * 512:q0 + (j + 1) * 512],
                        tile_position=(r0, 0))
                if kc in TRICK_KCS:
                    # bits = max(A*s', 0): clamp in f32, then saturating
                    # f32->i16 conversion; the bits ARE bf16(exp(s'-B_SHIFT))
                    e = ebufp.tile([128, 1024], I16, tag="e")
                    nc.vector.tensor_scalar(
                        out=e[:], in0=ps[:], scalar1=A_TRICK, scalar2=0.0,
                        op0=mybir.AluOpType.mult, op1=mybir.AluOpType.max)
                    srci = e[:].bitcast(BF16)
                else:
                    e = ebufp.tile([128, 1024], BF16, tag="e")
                    nc.scalar.activation(
                        e[:], ps[:], mybir.ActivationFunctionType.Exp,
                        bias=nbsh[:])
                    srci = e[:]
                et = etlp.tile([128, 1024], BF16, tag="et")
                eng = nc.gpsimd if kc in POOL_KCS else nc.vector
                eng.tensor_tensor(out=et[:], in0=srci,
                                  in1=mh_tiles[i][kc // 16][:, kc % 16, :],
                                  op=mybir.AluOpType.mult)
                ets[(i, h, kc)] = et
                # per-kc PV lag: Pool mask-mults are slow (2.1us), give their
                # PVs a deeper lag; kc=KT-1 (stop flag) must issue last.
                pv_pending.append((idx + pv_lag(kc), i, h, kc))
                ready = [p for p in pv_pending if p[0] <= idx]
                for p in sorted(ready, key=lambda p: p[3]):
                    if p[3] == KT - 1 and any(q[3] != KT - 1 and q[1:3] == p[1:3]
                                              for q in pv_pending):
                        continue
                    pv_pending.remove(p)
                    issue_pv(p[1], p[2], p[3])
            for p in sorted(pv_pending, key=lambda p: (p[1], p[2], p[3])):
                issue_pv(p[1], p[2], p[3])
            for i, _ in due_tails:
                emit_tail(i)

            for c in reversed(_mainctx):
                c.__exit__(None, None, None)
            mall_ctx.__exit__(None, None, None)

    nc.finalize()
    _CACHE[key] = nc
    return nc


def _prep_host(query, key, value, mask, w0, b0, w1, b1):
    # outc permutation: group gp = h//2 holds head 2gp at rows 0-31 and head
    # 2gp+1 at rows 64-95; rows 32-63/96-127 are zero padding (row 32/96 later
    # becomes the augmented shift row on device).
    w0p = np.zeros((D, 4 * 128), np.float32)
    b0v = np.zeros((128, 4), np.float32)
    inds = np.zeros((128, 4 * H), np.float32)
    indst = np.zeros((H, 4 * 128), np.float32)
    w0t = w0.T.astype(np.float32)            # [inc, outc]
    for h in range(H):
        gp, u = divmod(h, 2)
        dst = gp * 128 + 64 * u
        w0p[:, dst:dst + 32] = w0t[:, 32 * h:32 * h + 32]
        b0v[64 * u:64 * u + 32, gp] = b0[32 * h:32 * h + 32]
        inds[64 * u:64 * u + 32, gp * H + h] = 1.0
        indst[h, gp * 128 + 64 * u:gp * 128 + 64 * u + 32] = 1.0
    w1t8 = np.ascontiguousarray(w1[:H].T.astype(BF16NP))
    b18v = np.asarray(b1[:H], np.float32).reshape(H, 1)
    w0p = w0p.astype(BF16NP)
    inds = inds.astype(BF16NP)
    indst = indst.astype(BF16NP)
    qbf = query.astype(BF16NP)
    kbf = key.astype(BF16NP)
    mbits = mask.astype(BF16NP)              # {0, 1} multiplicative
    in_maps = []
    for c in range(NCORES):
        b, half = divmod(c, 2)
        r0 = half * R
        in_maps.append({
            "q": np.ascontiguousarray(qbf[b, r0:r0 + R]),
            "k": np.ascontiguousarray(kbf[b]),
            "v": np.ascontiguousarray(value[b].reshape(1, SK)),
            "mt": np.ascontiguousarray(mbits[b, r0:r0 + R].T),
            "w0p": w0p, "w1t8": w1t8, "b0v": b0v, "b18v": b18v,
            "inds": inds, "indst": indst,
        })
    return in_maps


def kernel(query, key, value, mask, w0, b0, w1, b1, _repeat=1):
    query = np.asarray(query, np.float32)
    key = np.asarray(key, np.float32)
    value = np.asarray(value, np.float32)
    mask = np.asarray(mask, np.int32)
    nc = _build(_repeat)
    in_maps = _prep_host(query, key, value, mask, w0, b0, w1, b1)
    res = bass_utils.run_bass_kernel_spmd(nc, in_maps, core_ids=list(range(NCORES)))
    out = np.empty((B, SQ, 1), np.float32)
    for c in range(NCORES):
        b, half = divmod(c, 2)
        out[b, half * R:(half + 1) * R, 0] = res.results[c]["o"].reshape(R)
    return out
